# revision 12
# baseline (speedup 1.0000x reference)
"""ColBERT pairwise + in-batch negative CE loss on 8 Trainium2 NeuronCores.

Problem shapes (hardcoded): B=64, N=32, S=256, D=128, fp32.

Per core: 8 docs (c dim sharded), all 2048 query rows. 16 "units" of
[128 q-rows x 8 docs x 256 s] fp32 scores land in PSUM (bufs=2) and must
leave through the only two PSUM-capable engines:
  - DVE reduce_max direct from PSUM  (~2.26us / unit)
  - ACT copy -> f16 arena (~1.9us) + DVE f16 tensor_max tree (~1.2us)
Drain total ~48 engine-us over 2 engines => ~24us balanced floor.

v2 layout vs baseline:
  - 7 consolidated input DMAs in priority order (dT first) across the
    vector/gpsimd/sync queues; scalar queue kept DMA-free so the implicit
    ACT_TABLE_LOAD runs during the preamble.
  - warm matmul chain keeps PE continuously busy from "main" start so the
    p-state ramp (3us continuous busy -> 2.4GHz) completes before real MMs.
  - drain plan: unit0 = halved DVE reduce (early vector work), units in
    V_MID get direct DVE reduces placed to fill tree gaps, the rest are
    ACT copies with f16 trees batched 3-4 units per tree; the last batch's
    stage-1 ops are emitted per-unit as copies land so only ~1.6us of tree
    work trails the final ACT copy.
  - no on-device n-sum: maxall [128,132] f16 is DMA'd out in two chunks
    (cols 0:64 mid-kernel, 64:132 at the end); the host does the n-sum,
    block remap, diag masking and softplus epilogue.

maxall layout: col block m (8 cols) = query chunk m's 8 doc maxes
(row p = q-row p of the chunk, col 8m+c = local doc c). cols 128/129 =
pairwise-neg maxes (local b = 4g+j at row 32j+n, col 128+g).
"""

import sys

import numpy as np


def _ensure_path():
    try:
        import concourse  # noqa: F401
    except ImportError:
        sys.path.insert(0, "/opt/trn_rl_repo")


_ensure_path()

import concourse.bacc as bacc  # noqa: E402
import concourse.mybir as mybir  # noqa: E402
from concourse.bass_utils import run_bass_kernel_spmd  # noqa: E402
from concourse.tile import TileContext  # noqa: E402

B, N, S, D = 64, 32, 256, 128
NC = 8
CL = B // NC  # docs / queries per core (8)
BN = B * N  # 2048 query rows
DCOLS = CL * S  # 2048 doc columns per core
NEG_INF_DIAG = 1000000.0

F32 = mybir.dt.float32
F16 = mybir.dt.float16
MMDT = mybir.dt.float16

_CACHE = {}

# ---- drain schedule ------------------------------------------------------
# unit index = query chunk m = maxall col block m.
A_SPLIT = 0         # A-unit whose copy is split in halves (earliest ACT op)
V_HEAD = 3          # halved direct reduce; late enough not to stall the
                    # PE/ACT phase-in with its PSUM-read WAR hazard
V_MID = (8, 13)     # direct DVE reduces slotted into tree gaps
TREE_BATCHES = [[0, 1, 2], [4, 5, 6], [7, 9, 10], [11, 12, 14], [15]]
A_UNITS = [m for b in TREE_BATCHES for m in b]
K_WARM = 5          # warm matmuls (1 low + rest mid ~ covers DMA phase)


def _install_ntff_shim():
    """Best-effort: register the axon NTFF profile hook so BASS_TRACE=1
    produces hardware profiles.  Safe no-op when unavailable."""
    try:
        import types

        import antenv

        if "antenv.axon_hooks" in sys.modules:
            return
        import trn_agent_boot.trn_boot as tb

        mod = types.ModuleType("antenv.axon_hooks")
        _hook = [None]
        mod.set_axon_ntff_profile_hook = lambda h: _hook.__setitem__(0, h)
        mod.get_axon_ntff_profile_hook = lambda: _hook[0]
        sys.modules["antenv.axon_hooks"] = mod
        antenv.axon_hooks = mod
        mod.set_axon_ntff_profile_hook(
            tb._ntff_profile_via_ctypes("/opt/axon/libaxon_pjrt.so")
        )
    except Exception:
        pass


def _build():
    nc = bacc.Bacc("TRN2", target_bir_lowering=False, debug=False, num_devices=NC)
    qT = nc.dram_tensor("qT", [D, BN], MMDT, kind="ExternalInput")
    dT = nc.dram_tensor("dT", [D, DCOLS], MMDT, kind="ExternalInput")
    nT = nc.dram_tensor("nT", [D, DCOLS], MMDT, kind="ExternalInput")
    qp = nc.dram_tensor("qp", [D, CL * N], MMDT, kind="ExternalInput")
    out_d = nc.dram_tensor("out", [128, 132], F16, kind="ExternalOutput")

    X = mybir.AxisListType.X

    with TileContext(nc) as tc:
        with (
            tc.tile_pool(name="sb", bufs=1) as sb,
            tc.tile_pool(name="ar", bufs=3) as arp,
            tc.tile_pool(name="tr", bufs=2) as trp,
            tc.tile_pool(name="ps", bufs=2, space="PSUM") as ps,
        ):
            qs = sb.tile([D, BN], MMDT, tag="qs")
            ds = sb.tile([D, DCOLS], MMDT, tag="ds")
            ns = sb.tile([D, DCOLS], MMDT, tag="ns")
            qps = sb.tile([D, CL * N], MMDT, tag="qps")
            maxall = sb.tile([128, 132], F16, tag="maxall")

            # Warm-up chain: memset-backed matmuls, no DMA dependency, keep
            # the PE continuously busy so the p-state ramp finishes before
            # the first real matmul.
            wa = sb.tile([D, 128], F16, tag="wa")
            wb = sb.tile([D, 512], F16, tag="wb")
            nc.gpsimd.memset(wa[:, :], 0.0)
            nc.gpsimd.memset(wb[:, :], 0.0)
            wt = ps.tile([128, 2048], F32, tag="chunk", name="warm")
            for w in range(K_WARM):
                nc.tensor.matmul(
                    wt[:, 512 * (w % 4) : 512 * (w % 4 + 1)],
                    wa[:, :],
                    wb[:, :],
                    start=True,
                    stop=True,
                )

            # Dummy 1-elem scalar copy: forces the implicit ACT_TABLE_LOAD
            # to be inserted here (runs during the DMA phase), so the first
            # real PSUM copy isn't delayed by the 1.28us table load.
            nc.scalar.copy(wb[0:1, 0:1], wa[0:1, 0:1])

            # Input DMAs, priority order.  The HWDGE queues (sync/SP and
            # scalar/ACT) are much faster to issue than gpsimd's SWDGE, so
            # the critical tensors (first query chunks + dT) go there; only
            # the tiny qp rides SWDGE.
            nc.sync.dma_start(out=qs[:, 0:512], in_=qT[:, 0:512])
            nc.scalar.dma_start(out=ds[:, 0:1024], in_=dT[:, 0:1024])
            nc.sync.dma_start(out=ds[:, 1024:2048], in_=dT[:, 1024:2048])
            nc.scalar.dma_start(out=qs[:, 512:1024], in_=qT[:, 512:1024])
            nc.sync.dma_start(out=ns[:, :], in_=nT[:, :])
            nc.gpsimd.dma_start(out=qps[:, :], in_=qp[:, :])
            nc.sync.dma_start(out=qs[:, 1024:2048], in_=qT[:, 1024:2048])

            arenas = {}
            bat_of = {}
            for bi, bb in enumerate(TREE_BATCHES):
                for sl, mm in enumerate(bb):
                    bat_of[mm] = (bi, sl)

            def get_arena(bi):
                if bi not in arenas:
                    arenas[bi] = arp.tile(
                        [128, 8192], F16, tag="arena", name=f"a{bi}"
                    )
                return arenas[bi]

            def emit_mms(m, t):
                bi, slot = bat_of.get(m, (None, None))
                for u in range(4):
                    nc.tensor.matmul(
                        t[:, 512 * u : 512 * (u + 1)],
                        qs[:, 128 * m : 128 * (m + 1)],
                        ds[:, 512 * u : 512 * (u + 1)],
                        start=True,
                        stop=True,
                    )
                    if m == V_HEAD and u == 1:
                        nc.vector.reduce_max(
                            maxall[:, 8 * m : 8 * m + 4],
                            t[:, 0:1024].rearrange("p (g s) -> p g s", s=S),
                            axis=X,
                        )
                    if m == A_SPLIT and u == 1:
                        nc.scalar.copy(
                            get_arena(bi)[:, 2048 * slot : 2048 * slot + 1024],
                            t[:, 0:1024],
                        )

            def emit_unit(m):
                t = ps.tile([128, 2048], F32, tag="chunk", name=f"u{m}")
                emit_mms(m, t)
                if m == V_HEAD:
                    nc.vector.reduce_max(
                        maxall[:, 8 * m + 4 : 8 * m + 8],
                        t[:, 1024:2048].rearrange("p (g s) -> p g s", s=S),
                        axis=X,
                    )
                elif m in V_MID:
                    # split in halves: shorter PSUM-read ops release the
                    # tile's WAR hazard sooner for the next unit's matmuls
                    nc.vector.reduce_max(
                        maxall[:, 8 * m : 8 * m + 4],
                        t[:, 0:1024].rearrange("p (g s) -> p g s", s=S),
                        axis=X,
                    )
                    nc.vector.reduce_max(
                        maxall[:, 8 * m + 4 : 8 * m + 8],
                        t[:, 1024:2048].rearrange("p (g s) -> p g s", s=S),
                        axis=X,
                    )
                elif m == A_SPLIT:
                    bi, slot = bat_of[m]
                    nc.scalar.copy(
                        get_arena(bi)[:, 2048 * slot + 1024 : 2048 * (slot + 1)],
                        t[:, 1024:2048],
                    )
                else:
                    bi, slot = bat_of[m]
                    nc.scalar.copy(
                        get_arena(bi)[:, 2048 * slot : 2048 * (slot + 1)], t[:, :]
                    )

            s1outs = {}

            def emit_tree_s1(bi, sl):
                # stage 1 for one unit of batch bi (per-unit so the last
                # batch can start as each copy lands)
                arena = arenas[bi]
                if bi not in s1outs:
                    L = len(TREE_BATCHES[bi])
                    s1outs[bi] = trp.tile([128, 1024 * L], F16, tag="t1",
                                          name=f"t1_{bi}")
                gv = arena[:, 2048 * sl : 2048 * (sl + 1)].rearrange(
                    "p (g s) -> p g s", s=256
                )
                ov = s1outs[bi][:, 1024 * sl : 1024 * (sl + 1)].rearrange(
                    "p (g s) -> p g s", s=128
                )
                nc.vector.tensor_max(ov, gv[:, :, 0:128], gv[:, :, 128:256])

            def emit_tree_rest(bi):
                L = len(TREE_BATCHES[bi])
                t1v = s1outs[bi][:, 0 : 1024 * L].rearrange(
                    "p (g s) -> p g s", s=128
                )
                t2 = trp.tile([128, 2048], F16, tag="t2")
                t2v = t2[:, 0 : 512 * L].rearrange("p (g s) -> p g s", s=64)
                nc.vector.tensor_max(t2v, t1v[:, :, 0:64], t1v[:, :, 64:128])
                t3 = trp.tile([128, 1024], F16, tag="t3")
                t3v = t3[:, 0 : 256 * L].rearrange("p (g s) -> p g s", s=32)
                nc.vector.tensor_max(t3v, t2v[:, :, 0:32], t2v[:, :, 32:64])
                t4 = trp.tile([128, 512], F16, tag="t4")
                t4v = t4[:, 0 : 128 * L].rearrange("p (g s) -> p g s", s=16)
                nc.vector.tensor_max(t4v, t3v[:, :, 0:16], t3v[:, :, 16:32])
                for k, mm in enumerate(TREE_BATCHES[bi]):
                    nc.vector.reduce_max(
                        maxall[:, 8 * mm : 8 * mm + 8],
                        t4v[:, 8 * k : 8 * (k + 1), :],
                        axis=X,
                    )

            def emit_tree(bi):
                for sl in range(len(TREE_BATCHES[bi])):
                    emit_tree_s1(bi, sl)
                emit_tree_rest(bi)

            def emit_pairwise():
                pt = ps.tile([128, 2048], F32, tag="chunk", name="ptw")
                for b in range(CL):
                    g, j = divmod(b, 4)
                    nc.tensor.matmul(
                        pt[32 * j : 32 * (j + 1), 256 * g : 256 * (g + 1)],
                        qps[:, 32 * b : 32 * (b + 1)],
                        ns[:, 256 * b : 256 * (b + 1)],
                        start=True,
                        stop=True,
                        tile_position=(0, 32 * j),
                    )
                nc.vector.reduce_max(
                    maxall[:, 128:130],
                    pt[:, 0:512].rearrange("p (g s) -> p g s", s=S),
                    axis=X,
                )

            # ---- emission order (PE + engine queues are FIFO) ----------
            emit_unit(0)          # A-split: ACT starts after 2 of 4 MMs
            emit_unit(1)          # A
            emit_unit(2)          # A
            emit_pairwise()       # early DVE filler (needs ns+qps)
            emit_unit(3)          # Vh: halved DVE reduce
            emit_tree(0)          # B0 {0,1,2}
            for m in (4, 5, 6):
                emit_unit(m)
            emit_tree(1)          # B1 {4,5,6}
            for m in (7, 8):      # A, V-mid
                emit_unit(m)
            for m in (9, 10):
                emit_unit(m)
            emit_tree(2)          # B2 {7,9,10}
            nc.sync.dma_start(out=out_d[:, 0:64], in_=maxall[:, 0:64])
            emit_unit(11)
            emit_tree_s1(3, 0)
            emit_unit(12)
            emit_tree_s1(3, 1)
            emit_unit(13)         # V-mid fills DVE while c14 runs
            emit_unit(14)
            emit_tree_s1(3, 2)
            emit_tree_rest(3)     # B3 {11,12,14} closes before last copy
            emit_unit(15)
            emit_tree(4)          # solo {15}: ~1.5us tail after last copy
            nc.sync.dma_start(out=out_d[:, 64:132], in_=maxall[:, 64:132])

    nc.finalize()
    return nc


LAST_RESULT = None


def kernel(query_embeddings, doc_embeddings, neg_doc_embeddings):
    global LAST_RESULT
    _install_ntff_shim()

    q = np.asarray(query_embeddings, dtype=np.float32)
    d = np.asarray(doc_embeddings, dtype=np.float32)
    g = np.asarray(neg_doc_embeddings, dtype=np.float32)
    assert q.shape == (B, N, D) and d.shape == (B, S, D) and g.shape == (B, S, D)

    qT_all = np.ascontiguousarray(
        q.transpose(2, 0, 1).reshape(D, BN).astype(np.float16)
    )

    in_maps = []
    for k in range(NC):
        dT_k = np.ascontiguousarray(
            d[CL * k : CL * (k + 1)].transpose(2, 0, 1).reshape(D, DCOLS)
            .astype(np.float16)
        )
        nT_k = np.ascontiguousarray(
            g[CL * k : CL * (k + 1)].transpose(2, 0, 1).reshape(D, DCOLS)
            .astype(np.float16)
        )
        qp_k = np.ascontiguousarray(qT_all[:, CL * N * k : CL * N * (k + 1)])
        in_maps.append({"qT": qT_all, "dT": dT_k, "nT": nT_k, "qp": qp_k})

    if "nc" not in _CACHE:
        _CACHE["nc"] = _build()
    res = run_bass_kernel_spmd(_CACHE["nc"], in_maps, core_ids=list(range(NC)))
    LAST_RESULT = res

    # Host epilogue: n-sum, assembly, softplus means.
    scores = np.empty((B, B), dtype=np.float64)
    negpair = np.empty((B,), dtype=np.float64)
    for k in range(NC):
        o = res.results[k]["out"].astype(np.float64)  # (128, 132)
        for m in range(16):
            # rows 32j+n, block col 8m+c  ->  scores[4m+j, CL*k+c]
            blk = o[:, 8 * m : 8 * m + 8].reshape(4, N, CL).sum(axis=1)
            scores[4 * m : 4 * m + 4, CL * k : CL * (k + 1)] = blk
        pw = o[:, 128:130].reshape(4, N, 2).sum(axis=1)  # rows j, col g
        for gcol in range(2):
            for j in range(4):
                negpair[CL * k + 4 * gcol + j] = pw[j, gcol]

    pos = np.diagonal(scores)
    l1 = np.logaddexp(0.0, negpair - pos).mean()
    neg_ib = (scores - np.eye(B, dtype=np.float64) * NEG_INF_DIAG).max(axis=1)
    l2 = np.logaddexp(0.0, neg_ib - pos).mean()
    return np.asarray((l1 + l2) / 2.0, dtype=np.float32)


# revision 13
# speedup vs baseline: 1.0156x; 1.0156x over previous
"""ColBERT pairwise + in-batch negative CE loss on 8 Trainium2 NeuronCores.

Problem shapes (hardcoded): B=64, N=32, S=256, D=128, fp32.

Per core: 8 docs (c dim sharded), all 2048 query rows. 16 "units" of
[128 q-rows x 8 docs x 256 s] fp32 scores land in PSUM (bufs=2) and must
leave through the only two PSUM-capable engines:
  - DVE reduce_max direct from PSUM  (~2.26us / unit)
  - ACT copy -> f16 arena (~1.9us) + DVE f16 tensor_max tree (~1.2us)
Drain total ~48 engine-us over 2 engines => ~24us balanced floor.

v2 layout vs baseline:
  - 7 consolidated input DMAs in priority order (dT first) across the
    vector/gpsimd/sync queues; scalar queue kept DMA-free so the implicit
    ACT_TABLE_LOAD runs during the preamble.
  - warm matmul chain keeps PE continuously busy from "main" start so the
    p-state ramp (3us continuous busy -> 2.4GHz) completes before real MMs.
  - drain plan: unit0 = halved DVE reduce (early vector work), units in
    V_MID get direct DVE reduces placed to fill tree gaps, the rest are
    ACT copies with f16 trees batched 3-4 units per tree; the last batch's
    stage-1 ops are emitted per-unit as copies land so only ~1.6us of tree
    work trails the final ACT copy.
  - no on-device n-sum: maxall [128,132] f16 is DMA'd out in two chunks
    (cols 0:64 mid-kernel, 64:132 at the end); the host does the n-sum,
    block remap, diag masking and softplus epilogue.

maxall layout: col block m (8 cols) = query chunk m's 8 doc maxes
(row p = q-row p of the chunk, col 8m+c = local doc c). cols 128/129 =
pairwise-neg maxes (local b = 4g+j at row 32j+n, col 128+g).
"""

import sys

import numpy as np


def _ensure_path():
    try:
        import concourse  # noqa: F401
    except ImportError:
        sys.path.insert(0, "/opt/trn_rl_repo")


_ensure_path()

import concourse.bacc as bacc  # noqa: E402
import concourse.mybir as mybir  # noqa: E402
from concourse.bass_utils import run_bass_kernel_spmd  # noqa: E402
from concourse.tile import TileContext  # noqa: E402

B, N, S, D = 64, 32, 256, 128
NC = 8
CL = B // NC  # docs / queries per core (8)
BN = B * N  # 2048 query rows
DCOLS = CL * S  # 2048 doc columns per core
NEG_INF_DIAG = 1000000.0

F32 = mybir.dt.float32
F16 = mybir.dt.float16
MMDT = mybir.dt.float16

_CACHE = {}

# ---- drain schedule ------------------------------------------------------
# unit index = query chunk m = maxall col block m.
A_SPLIT = 0         # A-unit whose copy is split in halves (earliest ACT op)
V_HEAD = 3          # halved direct reduce; late enough not to stall the
                    # PE/ACT phase-in with its PSUM-read WAR hazard
V_MID = (8, 13)     # direct DVE reduces slotted into tree gaps
TREE_BATCHES = [[0, 1, 2], [4, 5, 6], [7, 9, 10], [11, 12, 14], [15]]
A_UNITS = [m for b in TREE_BATCHES for m in b]
K_WARM = 5          # warm matmuls (1 low + rest mid ~ covers DMA phase)


def _install_ntff_shim():
    """Best-effort: register the axon NTFF profile hook so BASS_TRACE=1
    produces hardware profiles.  Safe no-op when unavailable."""
    try:
        import types

        import antenv

        if "antenv.axon_hooks" in sys.modules:
            return
        import trn_agent_boot.trn_boot as tb

        mod = types.ModuleType("antenv.axon_hooks")
        _hook = [None]
        mod.set_axon_ntff_profile_hook = lambda h: _hook.__setitem__(0, h)
        mod.get_axon_ntff_profile_hook = lambda: _hook[0]
        sys.modules["antenv.axon_hooks"] = mod
        antenv.axon_hooks = mod
        mod.set_axon_ntff_profile_hook(
            tb._ntff_profile_via_ctypes("/opt/axon/libaxon_pjrt.so")
        )
    except Exception:
        pass


def _build():
    nc = bacc.Bacc("TRN2", target_bir_lowering=False, debug=False, num_devices=NC)
    qT = nc.dram_tensor("qT", [D, BN], MMDT, kind="ExternalInput")
    dT = nc.dram_tensor("dT", [D, DCOLS], MMDT, kind="ExternalInput")
    nT = nc.dram_tensor("nT", [D, DCOLS], MMDT, kind="ExternalInput")
    qp = nc.dram_tensor("qp", [D, CL * N], MMDT, kind="ExternalInput")
    out_d = nc.dram_tensor("out", [128, 132], F16, kind="ExternalOutput")

    X = mybir.AxisListType.X

    with TileContext(nc) as tc:
        with (
            tc.tile_pool(name="sb", bufs=1) as sb,
            tc.tile_pool(name="ar", bufs=3) as arp,
            tc.tile_pool(name="tr", bufs=2) as trp,
            tc.tile_pool(name="ps", bufs=2, space="PSUM") as ps,
        ):
            qs = sb.tile([D, BN], MMDT, tag="qs")
            ds = sb.tile([D, DCOLS], MMDT, tag="ds")
            ns = sb.tile([D, DCOLS], MMDT, tag="ns")
            qps = sb.tile([D, CL * N], MMDT, tag="qps")
            maxall = sb.tile([128, 132], F16, tag="maxall")

            # Warm-up chain: memset-backed matmuls, no DMA dependency, keep
            # the PE continuously busy so the p-state ramp finishes before
            # the first real matmul.
            wa = sb.tile([D, 128], F16, tag="wa")
            wb = sb.tile([D, 512], F16, tag="wb")
            nc.gpsimd.memset(wa[:, :], 0.0)
            nc.gpsimd.memset(wb[:, :], 0.0)
            wt = ps.tile([128, 2048], F32, tag="chunk", name="warm")
            for w in range(K_WARM):
                nc.tensor.matmul(
                    wt[:, 512 * (w % 4) : 512 * (w % 4 + 1)],
                    wa[:, :],
                    wb[:, :],
                    start=True,
                    stop=True,
                )

            # Dummy 1-elem scalar copy: forces the implicit ACT_TABLE_LOAD
            # to be inserted here (runs during the DMA phase), so the first
            # real PSUM copy isn't delayed by the 1.28us table load.
            nc.scalar.copy(wb[0:1, 0:1], wa[0:1, 0:1])

            # Input DMAs, priority order.  The HWDGE queues (sync/SP and
            # scalar/ACT) are much faster to issue than gpsimd's SWDGE, so
            # the critical tensors (first query chunks + dT) go there; only
            # the tiny qp rides SWDGE.
            nc.sync.dma_start(out=qs[:, 0:512], in_=qT[:, 0:512])
            nc.scalar.dma_start(out=ds[:, 0:1024], in_=dT[:, 0:1024])
            nc.sync.dma_start(out=ds[:, 1024:2048], in_=dT[:, 1024:2048])
            nc.scalar.dma_start(out=qs[:, 512:1024], in_=qT[:, 512:1024])
            nc.sync.dma_start(out=ns[:, :], in_=nT[:, :])
            nc.gpsimd.dma_start(out=qps[:, :], in_=qp[:, :])
            nc.sync.dma_start(out=qs[:, 1024:2048], in_=qT[:, 1024:2048])

            arenas = {}
            bat_of = {}
            for bi, bb in enumerate(TREE_BATCHES):
                for sl, mm in enumerate(bb):
                    bat_of[mm] = (bi, sl)

            def get_arena(bi):
                if bi not in arenas:
                    arenas[bi] = arp.tile(
                        [128, 8192], F16, tag="arena", name=f"a{bi}"
                    )
                return arenas[bi]

            def emit_mms(m, t):
                bi, slot = bat_of.get(m, (None, None))
                for u in range(4):
                    nc.tensor.matmul(
                        t[:, 512 * u : 512 * (u + 1)],
                        qs[:, 128 * m : 128 * (m + 1)],
                        ds[:, 512 * u : 512 * (u + 1)],
                        start=True,
                        stop=True,
                    )
                    if m == V_HEAD and u == 1:
                        nc.vector.reduce_max(
                            maxall[:, 8 * m : 8 * m + 4],
                            t[:, 0:1024].rearrange("p (g s) -> p g s", s=S),
                            axis=X,
                        )
                    if m == A_SPLIT and u == 1:
                        nc.scalar.copy(
                            get_arena(bi)[:, 2048 * slot : 2048 * slot + 1024],
                            t[:, 0:1024],
                        )

            def emit_unit(m):
                t = ps.tile([128, 2048], F32, tag="chunk", name=f"u{m}")
                emit_mms(m, t)
                if m == V_HEAD:
                    nc.vector.reduce_max(
                        maxall[:, 8 * m + 4 : 8 * m + 8],
                        t[:, 1024:2048].rearrange("p (g s) -> p g s", s=S),
                        axis=X,
                    )
                elif m in V_MID:
                    nc.vector.reduce_max(
                        maxall[:, 8 * m : 8 * m + 8],
                        t[:, :].rearrange("p (g s) -> p g s", s=S),
                        axis=X,
                    )
                elif m == A_SPLIT:
                    bi, slot = bat_of[m]
                    nc.scalar.copy(
                        get_arena(bi)[:, 2048 * slot + 1024 : 2048 * (slot + 1)],
                        t[:, 1024:2048],
                    )
                else:
                    bi, slot = bat_of[m]
                    nc.scalar.copy(
                        get_arena(bi)[:, 2048 * slot : 2048 * (slot + 1)], t[:, :]
                    )

            s1outs = {}

            def emit_tree_s1(bi, sl):
                # stage 1 for one unit of batch bi (per-unit so the last
                # batch can start as each copy lands)
                arena = arenas[bi]
                if bi not in s1outs:
                    L = len(TREE_BATCHES[bi])
                    s1outs[bi] = trp.tile([128, 1024 * L], F16, tag="t1",
                                          name=f"t1_{bi}")
                gv = arena[:, 2048 * sl : 2048 * (sl + 1)].rearrange(
                    "p (g s) -> p g s", s=256
                )
                ov = s1outs[bi][:, 1024 * sl : 1024 * (sl + 1)].rearrange(
                    "p (g s) -> p g s", s=128
                )
                nc.vector.tensor_max(ov, gv[:, :, 0:128], gv[:, :, 128:256])

            def emit_tree_rest(bi):
                L = len(TREE_BATCHES[bi])
                t1v = s1outs[bi][:, 0 : 1024 * L].rearrange(
                    "p (g s) -> p g s", s=128
                )
                t2 = trp.tile([128, 2048], F16, tag="t2")
                t2v = t2[:, 0 : 512 * L].rearrange("p (g s) -> p g s", s=64)
                nc.vector.tensor_max(t2v, t1v[:, :, 0:64], t1v[:, :, 64:128])
                t3 = trp.tile([128, 1024], F16, tag="t3")
                t3v = t3[:, 0 : 256 * L].rearrange("p (g s) -> p g s", s=32)
                nc.vector.tensor_max(t3v, t2v[:, :, 0:32], t2v[:, :, 32:64])
                t4 = trp.tile([128, 512], F16, tag="t4")
                t4v = t4[:, 0 : 128 * L].rearrange("p (g s) -> p g s", s=16)
                nc.vector.tensor_max(t4v, t3v[:, :, 0:16], t3v[:, :, 16:32])
                for k, mm in enumerate(TREE_BATCHES[bi]):
                    nc.vector.reduce_max(
                        maxall[:, 8 * mm : 8 * mm + 8],
                        t4v[:, 8 * k : 8 * (k + 1), :],
                        axis=X,
                    )

            def emit_tree(bi):
                for sl in range(len(TREE_BATCHES[bi])):
                    emit_tree_s1(bi, sl)
                emit_tree_rest(bi)

            def emit_pairwise():
                pt = ps.tile([128, 2048], F32, tag="chunk", name="ptw")
                for b in range(CL):
                    g, j = divmod(b, 4)
                    nc.tensor.matmul(
                        pt[32 * j : 32 * (j + 1), 256 * g : 256 * (g + 1)],
                        qps[:, 32 * b : 32 * (b + 1)],
                        ns[:, 256 * b : 256 * (b + 1)],
                        start=True,
                        stop=True,
                        tile_position=(0, 32 * j),
                    )
                nc.vector.reduce_max(
                    maxall[:, 128:130],
                    pt[:, 0:512].rearrange("p (g s) -> p g s", s=S),
                    axis=X,
                )

            # ---- emission order (PE + engine queues are FIFO) ----------
            emit_unit(0)          # A-split: ACT starts after 2 of 4 MMs
            emit_unit(1)          # A
            emit_unit(2)          # A
            emit_pairwise()       # early DVE filler (needs ns+qps)
            emit_unit(3)          # Vh: halved DVE reduce
            emit_tree(0)          # B0 {0,1,2}
            for m in (4, 5, 6):
                emit_unit(m)
            emit_tree(1)          # B1 {4,5,6}
            for m in (7, 8):      # A, V-mid
                emit_unit(m)
            for m in (9, 10):
                emit_unit(m)
            emit_tree(2)          # B2 {7,9,10}
            nc.sync.dma_start(out=out_d[:, 0:64], in_=maxall[:, 0:64])
            emit_unit(11)
            emit_tree_s1(3, 0)
            emit_unit(12)
            emit_tree_s1(3, 1)
            emit_unit(13)         # V-mid fills DVE while c14 runs
            emit_unit(14)
            emit_tree_s1(3, 2)
            emit_tree_rest(3)     # B3 {11,12,14} closes before last copy
            emit_unit(15)
            emit_tree(4)          # solo {15}: ~1.5us tail after last copy
            nc.sync.dma_start(out=out_d[:, 64:132], in_=maxall[:, 64:132])

    nc.finalize()
    return nc


LAST_RESULT = None


def kernel(query_embeddings, doc_embeddings, neg_doc_embeddings):
    global LAST_RESULT
    _install_ntff_shim()

    q = np.asarray(query_embeddings, dtype=np.float32)
    d = np.asarray(doc_embeddings, dtype=np.float32)
    g = np.asarray(neg_doc_embeddings, dtype=np.float32)
    assert q.shape == (B, N, D) and d.shape == (B, S, D) and g.shape == (B, S, D)

    qT_all = np.ascontiguousarray(
        q.transpose(2, 0, 1).reshape(D, BN).astype(np.float16)
    )

    in_maps = []
    for k in range(NC):
        dT_k = np.ascontiguousarray(
            d[CL * k : CL * (k + 1)].transpose(2, 0, 1).reshape(D, DCOLS)
            .astype(np.float16)
        )
        nT_k = np.ascontiguousarray(
            g[CL * k : CL * (k + 1)].transpose(2, 0, 1).reshape(D, DCOLS)
            .astype(np.float16)
        )
        qp_k = np.ascontiguousarray(qT_all[:, CL * N * k : CL * N * (k + 1)])
        in_maps.append({"qT": qT_all, "dT": dT_k, "nT": nT_k, "qp": qp_k})

    if "nc" not in _CACHE:
        _CACHE["nc"] = _build()
    res = run_bass_kernel_spmd(_CACHE["nc"], in_maps, core_ids=list(range(NC)))
    LAST_RESULT = res

    # Host epilogue: n-sum, assembly, softplus means.
    scores = np.empty((B, B), dtype=np.float64)
    negpair = np.empty((B,), dtype=np.float64)
    for k in range(NC):
        o = res.results[k]["out"].astype(np.float64)  # (128, 132)
        for m in range(16):
            # rows 32j+n, block col 8m+c  ->  scores[4m+j, CL*k+c]
            blk = o[:, 8 * m : 8 * m + 8].reshape(4, N, CL).sum(axis=1)
            scores[4 * m : 4 * m + 4, CL * k : CL * (k + 1)] = blk
        pw = o[:, 128:130].reshape(4, N, 2).sum(axis=1)  # rows j, col g
        for gcol in range(2):
            for j in range(4):
                negpair[CL * k + 4 * gcol + j] = pw[j, gcol]

    pos = np.diagonal(scores)
    l1 = np.logaddexp(0.0, negpair - pos).mean()
    neg_ib = (scores - np.eye(B, dtype=np.float64) * NEG_INF_DIAG).max(axis=1)
    l2 = np.logaddexp(0.0, neg_ib - pos).mean()
    return np.asarray((l1 + l2) / 2.0, dtype=np.float32)


# revision 18
# speedup vs baseline: 1.0174x; 1.0017x over previous
"""ColBERT pairwise + in-batch negative CE loss on 8 Trainium2 NeuronCores.

Problem shapes (hardcoded): B=64, N=32, S=256, D=128, fp32.

Per core: 8 docs (c dim sharded), all 2048 query rows. 16 "units" of
[128 q-rows x 8 docs x 256 s] fp32 scores land in PSUM (bufs=2) and must
leave through the only two PSUM-capable engines:
  - DVE reduce_max direct from PSUM  (~2.26us / unit)
  - ACT copy -> f16 arena (~1.9us) + DVE f16 tensor_max tree (~1.2us)
Drain total ~48 engine-us over 2 engines => ~24us balanced floor.

v2 layout vs baseline:
  - 7 consolidated input DMAs in priority order (dT first) across the
    vector/gpsimd/sync queues; scalar queue kept DMA-free so the implicit
    ACT_TABLE_LOAD runs during the preamble.
  - warm matmul chain keeps PE continuously busy from "main" start so the
    p-state ramp (3us continuous busy -> 2.4GHz) completes before real MMs.
  - drain plan: unit0 = halved DVE reduce (early vector work), units in
    V_MID get direct DVE reduces placed to fill tree gaps, the rest are
    ACT copies with f16 trees batched 3-4 units per tree; the last batch's
    stage-1 ops are emitted per-unit as copies land so only ~1.6us of tree
    work trails the final ACT copy.
  - no on-device n-sum: maxall [128,132] f16 is DMA'd out in two chunks
    (cols 0:64 mid-kernel, 64:132 at the end); the host does the n-sum,
    block remap, diag masking and softplus epilogue.

maxall layout: col block m (8 cols) = query chunk m's 8 doc maxes
(row p = q-row p of the chunk, col 8m+c = local doc c). cols 128/129 =
pairwise-neg maxes (local b = 4g+j at row 32j+n, col 128+g).
"""

import sys

import numpy as np


def _ensure_path():
    try:
        import concourse  # noqa: F401
    except ImportError:
        sys.path.insert(0, "/opt/trn_rl_repo")


_ensure_path()

import concourse.bacc as bacc  # noqa: E402
import concourse.mybir as mybir  # noqa: E402
from concourse.bass_utils import run_bass_kernel_spmd  # noqa: E402
from concourse.tile import TileContext  # noqa: E402

B, N, S, D = 64, 32, 256, 128
NC = 8
CL = B // NC  # docs / queries per core (8)
BN = B * N  # 2048 query rows
DCOLS = CL * S  # 2048 doc columns per core
NEG_INF_DIAG = 1000000.0

F32 = mybir.dt.float32
F16 = mybir.dt.float16
MMDT = mybir.dt.float16

_CACHE = {}

# ---- drain schedule ------------------------------------------------------
# unit index = query chunk m = maxall col block m.
A_SPLIT = 0         # A-unit whose copy is split in halves (earliest ACT op)
V_HEAD = 3          # halved direct reduce; late enough not to stall the
                    # PE/ACT phase-in with its PSUM-read WAR hazard
V_MID = (8, 13)     # direct DVE reduces slotted into tree gaps
TREE_BATCHES = [[0, 1, 2], [4, 5, 6], [7, 9, 10], [11, 12, 14], [15]]
A_UNITS = [m for b in TREE_BATCHES for m in b]
K_WARM = 5          # warm matmuls (1 low + rest mid ~ covers DMA phase)


def _install_ntff_shim():
    """Best-effort: register the axon NTFF profile hook so BASS_TRACE=1
    produces hardware profiles.  Safe no-op when unavailable."""
    try:
        import types

        import antenv

        if "antenv.axon_hooks" in sys.modules:
            return
        import trn_agent_boot.trn_boot as tb

        mod = types.ModuleType("antenv.axon_hooks")
        _hook = [None]
        mod.set_axon_ntff_profile_hook = lambda h: _hook.__setitem__(0, h)
        mod.get_axon_ntff_profile_hook = lambda: _hook[0]
        sys.modules["antenv.axon_hooks"] = mod
        antenv.axon_hooks = mod
        mod.set_axon_ntff_profile_hook(
            tb._ntff_profile_via_ctypes("/opt/axon/libaxon_pjrt.so")
        )
    except Exception:
        pass


def _build():
    nc = bacc.Bacc("TRN2", target_bir_lowering=False, debug=False, num_devices=NC)
    qT = nc.dram_tensor("qT", [D, BN], MMDT, kind="ExternalInput")
    dT = nc.dram_tensor("dT", [D, DCOLS], MMDT, kind="ExternalInput")
    nT = nc.dram_tensor("nT", [D, DCOLS], MMDT, kind="ExternalInput")
    qp = nc.dram_tensor("qp", [D, CL * N], MMDT, kind="ExternalInput")
    out_d = nc.dram_tensor("out", [128, 132], F16, kind="ExternalOutput")

    X = mybir.AxisListType.X

    with TileContext(nc) as tc:
        with (
            tc.tile_pool(name="sb", bufs=1) as sb,
            tc.tile_pool(name="ar", bufs=3) as arp,
            tc.tile_pool(name="tr", bufs=2) as trp,
            tc.tile_pool(name="ps", bufs=2, space="PSUM") as ps,
        ):
            qs = sb.tile([D, BN], MMDT, tag="qs")
            ds = sb.tile([D, DCOLS], MMDT, tag="ds")
            ns = sb.tile([D, DCOLS], MMDT, tag="ns")
            qps = sb.tile([D, CL * N], MMDT, tag="qps")
            maxall = sb.tile([128, 132], F16, tag="maxall")

            # Warm-up chain: memset-backed matmuls, no DMA dependency, keep
            # the PE continuously busy so the p-state ramp finishes before
            # the first real matmul.
            wa = sb.tile([D, 128], F16, tag="wa")
            wb = sb.tile([D, 512], F16, tag="wb")
            nc.gpsimd.memset(wa[:, :], 0.0)
            nc.gpsimd.memset(wb[:, :], 0.0)
            wt = ps.tile([128, 2048], F32, tag="chunk", name="warm")
            for w in range(K_WARM):
                nc.tensor.matmul(
                    wt[:, 512 * (w % 4) : 512 * (w % 4 + 1)],
                    wa[:, :],
                    wb[:, :],
                    start=True,
                    stop=True,
                )

            # Dummy 1-elem scalar copy: forces the implicit ACT_TABLE_LOAD
            # to be inserted here (runs during the DMA phase), so the first
            # real PSUM copy isn't delayed by the 1.28us table load.
            nc.scalar.copy(wb[0:1, 0:1], wa[0:1, 0:1])

            # Input DMAs, priority order.  The HWDGE queues (sync/SP and
            # scalar/ACT) are much faster to issue than gpsimd's SWDGE, so
            # the critical tensors (first query chunks + dT) go there; only
            # the tiny qp rides SWDGE.
            nc.sync.dma_start(out=qs[:, 0:512], in_=qT[:, 0:512])
            nc.scalar.dma_start(out=ds[:, 0:1024], in_=dT[:, 0:1024])
            nc.sync.dma_start(out=ds[:, 1024:2048], in_=dT[:, 1024:2048])
            nc.scalar.dma_start(out=qs[:, 512:1024], in_=qT[:, 512:1024])
            nc.sync.dma_start(out=ns[:, :], in_=nT[:, :])
            nc.gpsimd.dma_start(out=qps[:, :], in_=qp[:, :])
            nc.sync.dma_start(out=qs[:, 1024:2048], in_=qT[:, 1024:2048])

            arenas = {}
            bat_of = {}
            for bi, bb in enumerate(TREE_BATCHES):
                for sl, mm in enumerate(bb):
                    bat_of[mm] = (bi, sl)

            def get_arena(bi):
                if bi not in arenas:
                    arenas[bi] = arp.tile(
                        [128, 8192], F16, tag="arena", name=f"a{bi}"
                    )
                return arenas[bi]

            def emit_mms(m, t):
                bi, slot = bat_of.get(m, (None, None))
                for u in range(4):
                    nc.tensor.matmul(
                        t[:, 512 * u : 512 * (u + 1)],
                        qs[:, 128 * m : 128 * (m + 1)],
                        ds[:, 512 * u : 512 * (u + 1)],
                        start=True,
                        stop=True,
                    )
                    if m == V_HEAD and u == 1:
                        nc.vector.reduce_max(
                            maxall[:, 8 * m : 8 * m + 4],
                            t[:, 0:1024].rearrange("p (g s) -> p g s", s=S),
                            axis=X,
                        )
                    if m == A_SPLIT and u == 1:
                        nc.scalar.copy(
                            get_arena(bi)[:, 2048 * slot : 2048 * slot + 1024],
                            t[:, 0:1024],
                        )

            def emit_unit(m):
                t = ps.tile([128, 2048], F32, tag="chunk", name=f"u{m}")
                emit_mms(m, t)
                if m == V_HEAD:
                    nc.vector.reduce_max(
                        maxall[:, 8 * m + 4 : 8 * m + 8],
                        t[:, 1024:2048].rearrange("p (g s) -> p g s", s=S),
                        axis=X,
                    )
                elif m in V_MID:
                    nc.vector.reduce_max(
                        maxall[:, 8 * m : 8 * m + 8],
                        t[:, :].rearrange("p (g s) -> p g s", s=S),
                        axis=X,
                    )
                elif m == A_SPLIT:
                    bi, slot = bat_of[m]
                    nc.scalar.copy(
                        get_arena(bi)[:, 2048 * slot + 1024 : 2048 * (slot + 1)],
                        t[:, 1024:2048],
                    )
                else:
                    bi, slot = bat_of[m]
                    nc.scalar.copy(
                        get_arena(bi)[:, 2048 * slot : 2048 * (slot + 1)], t[:, :]
                    )

            s1outs = {}

            def emit_tree_s1(bi, sl):
                # stage 1 for one unit of batch bi (per-unit so the last
                # batch can start as each copy lands)
                arena = arenas[bi]
                if bi not in s1outs:
                    L = len(TREE_BATCHES[bi])
                    s1outs[bi] = trp.tile([128, 1024 * L], F16, tag="t1",
                                          name=f"t1_{bi}")
                gv = arena[:, 2048 * sl : 2048 * (sl + 1)].rearrange(
                    "p (g s) -> p g s", s=256
                )
                ov = s1outs[bi][:, 1024 * sl : 1024 * (sl + 1)].rearrange(
                    "p (g s) -> p g s", s=128
                )
                nc.vector.tensor_max(ov, gv[:, :, 0:128], gv[:, :, 128:256])

            def emit_tree_rest(bi):
                L = len(TREE_BATCHES[bi])
                t1v = s1outs[bi][:, 0 : 1024 * L].rearrange(
                    "p (g s) -> p g s", s=128
                )
                t2 = trp.tile([128, 2048], F16, tag="t2")
                t2v = t2[:, 0 : 512 * L].rearrange("p (g s) -> p g s", s=64)
                nc.vector.tensor_max(t2v, t1v[:, :, 0:64], t1v[:, :, 64:128])
                t3 = trp.tile([128, 1024], F16, tag="t3")
                t3v = t3[:, 0 : 256 * L].rearrange("p (g s) -> p g s", s=32)
                nc.vector.tensor_max(t3v, t2v[:, :, 0:32], t2v[:, :, 32:64])
                t4 = trp.tile([128, 512], F16, tag="t4")
                t4v = t4[:, 0 : 128 * L].rearrange("p (g s) -> p g s", s=16)
                nc.vector.tensor_max(t4v, t3v[:, :, 0:16], t3v[:, :, 16:32])
                for k, mm in enumerate(TREE_BATCHES[bi]):
                    nc.vector.reduce_max(
                        maxall[:, 8 * mm : 8 * mm + 8],
                        t4v[:, 8 * k : 8 * (k + 1), :],
                        axis=X,
                    )

            def emit_tree(bi):
                for sl in range(len(TREE_BATCHES[bi])):
                    emit_tree_s1(bi, sl)
                emit_tree_rest(bi)

            def emit_pairwise():
                pt = ps.tile([128, 2048], F32, tag="chunk", name="ptw")
                for b in range(CL):
                    g, j = divmod(b, 4)
                    nc.tensor.matmul(
                        pt[32 * j : 32 * (j + 1), 256 * g : 256 * (g + 1)],
                        qps[:, 32 * b : 32 * (b + 1)],
                        ns[:, 256 * b : 256 * (b + 1)],
                        start=True,
                        stop=True,
                        tile_position=(0, 32 * j),
                    )
                nc.vector.reduce_max(
                    maxall[:, 128:130],
                    pt[:, 0:512].rearrange("p (g s) -> p g s", s=S),
                    axis=X,
                )

            # ---- emission order (PE + engine queues are FIFO) ----------
            emit_unit(0)          # A-split: ACT starts after 2 of 4 MMs
            emit_unit(1)          # A
            emit_unit(2)          # A
            emit_pairwise()       # early DVE filler (needs ns+qps)
            emit_unit(3)          # Vh: halved DVE reduce
            emit_tree(0)          # B0 {0,1,2}
            for m in (4, 5, 6):
                emit_unit(m)
            emit_tree(1)          # B1 {4,5,6}
            for m in (7, 8):      # A, V-mid
                emit_unit(m)
            for m in (9, 10):
                emit_unit(m)
            emit_tree(2)          # B2 {7,9,10}
            nc.sync.dma_start(out=out_d[:, 0:64], in_=maxall[:, 0:64])
            emit_unit(11)
            emit_tree_s1(3, 0)
            emit_unit(12)
            emit_tree_s1(3, 1)
            emit_unit(13)         # V-mid fills DVE while c14 runs
            emit_unit(14)
            emit_tree_s1(3, 2)
            emit_tree_rest(3)     # B3 {11,12,14} closes before last copy
            emit_unit(15)
            emit_tree(4)          # solo {15}: ~1.5us tail after last copy
            nc.sync.dma_start(out=out_d[:, 64:132], in_=maxall[:, 64:132])

    nc.finalize()
    return nc


LAST_RESULT = None


def kernel(query_embeddings, doc_embeddings, neg_doc_embeddings):
    global LAST_RESULT
    _install_ntff_shim()

    q = np.asarray(query_embeddings, dtype=np.float32)
    d = np.asarray(doc_embeddings, dtype=np.float32)
    g = np.asarray(neg_doc_embeddings, dtype=np.float32)
    assert q.shape == (B, N, D) and d.shape == (B, S, D) and g.shape == (B, S, D)

    qT_all = np.ascontiguousarray(
        q.transpose(2, 0, 1).reshape(D, BN).astype(np.float16)
    )

    in_maps = []
    for k in range(NC):
        dT_k = np.ascontiguousarray(
            d[CL * k : CL * (k + 1)].transpose(2, 0, 1).reshape(D, DCOLS)
            .astype(np.float16)
        )
        nT_k = np.ascontiguousarray(
            g[CL * k : CL * (k + 1)].transpose(2, 0, 1).reshape(D, DCOLS)
            .astype(np.float16)
        )
        qp_k = np.ascontiguousarray(qT_all[:, CL * N * k : CL * N * (k + 1)])
        in_maps.append({"qT": qT_all, "dT": dT_k, "nT": nT_k, "qp": qp_k})

    if "nc" not in _CACHE:
        _CACHE["nc"] = _build()
    res = run_bass_kernel_spmd(_CACHE["nc"], in_maps, core_ids=list(range(NC)))
    LAST_RESULT = res

    # Host epilogue: n-sum, assembly, softplus means.
    scores = np.empty((B, B), dtype=np.float64)
    negpair = np.empty((B,), dtype=np.float64)
    for k in range(NC):
        o = res.results[k]["out"].astype(np.float64)  # (128, 132)
        for m in range(16):
            # rows 32j+n, block col 8m+c  ->  scores[4m+j, CL*k+c]
            blk = o[:, 8 * m : 8 * m + 8].reshape(4, N, CL).sum(axis=1)
            scores[4 * m : 4 * m + 4, CL * k : CL * (k + 1)] = blk
        pw = o[:, 128:130].reshape(4, N, 2).sum(axis=1)  # rows j, col g
        for gcol in range(2):
            for j in range(4):
                negpair[CL * k + 4 * gcol + j] = pw[j, gcol]

    pos = np.diagonal(scores)
    l1 = np.logaddexp(0.0, negpair - pos).mean()
    neg_ib = (scores - np.eye(B, dtype=np.float64) * NEG_INF_DIAG).max(axis=1)
    l2 = np.logaddexp(0.0, neg_ib - pos).mean()
    return np.asarray((l1 + l2) / 2.0, dtype=np.float32)


# revision 19
# speedup vs baseline: 1.0250x; 1.0075x over previous
"""ColBERT pairwise + in-batch negative CE loss on 8 Trainium2 NeuronCores.

Problem shapes (hardcoded): B=64, N=32, S=256, D=128, fp32.

Per core: 8 docs (c dim sharded), all 2048 query rows. 16 "units" of
[128 q-rows x 8 docs x 256 s] fp32 scores land in PSUM (bufs=2) and must
leave through the only two PSUM-capable engines:
  - DVE reduce_max direct from PSUM  (~2.26us / unit)
  - ACT copy -> f16 arena (~1.9us) + DVE f16 tensor_max tree (~1.2us)
Drain total ~48 engine-us over 2 engines => ~24us balanced floor.

v2 layout vs baseline:
  - 7 consolidated input DMAs in priority order (dT first) across the
    vector/gpsimd/sync queues; scalar queue kept DMA-free so the implicit
    ACT_TABLE_LOAD runs during the preamble.
  - warm matmul chain keeps PE continuously busy from "main" start so the
    p-state ramp (3us continuous busy -> 2.4GHz) completes before real MMs.
  - drain plan: unit0 = halved DVE reduce (early vector work), units in
    V_MID get direct DVE reduces placed to fill tree gaps, the rest are
    ACT copies with f16 trees batched 3-4 units per tree; the last batch's
    stage-1 ops are emitted per-unit as copies land so only ~1.6us of tree
    work trails the final ACT copy.
  - no on-device n-sum: maxall [128,132] f16 is DMA'd out in two chunks
    (cols 0:64 mid-kernel, 64:132 at the end); the host does the n-sum,
    block remap, diag masking and softplus epilogue.

maxall layout: col block m (8 cols) = query chunk m's 8 doc maxes
(row p = q-row p of the chunk, col 8m+c = local doc c). cols 128/129 =
pairwise-neg maxes (local b = 4g+j at row 32j+n, col 128+g).
"""

import sys

import numpy as np


def _ensure_path():
    try:
        import concourse  # noqa: F401
    except ImportError:
        sys.path.insert(0, "/opt/trn_rl_repo")


_ensure_path()

import concourse.bacc as bacc  # noqa: E402
import concourse.mybir as mybir  # noqa: E402
from concourse.bass_utils import run_bass_kernel_spmd  # noqa: E402
from concourse.tile import TileContext  # noqa: E402

B, N, S, D = 64, 32, 256, 128
NC = 8
CL = B // NC  # docs / queries per core (8)
BN = B * N  # 2048 query rows
DCOLS = CL * S  # 2048 doc columns per core
NEG_INF_DIAG = 1000000.0

F32 = mybir.dt.float32
F16 = mybir.dt.float16
MMDT = mybir.dt.float16

_CACHE = {}

# ---- drain schedule ------------------------------------------------------
# unit index = query chunk m = maxall col block m.
A_SPLIT = 0         # A-unit whose copy is split in halves (earliest ACT op)
V_HEAD = 3          # halved direct reduce; late enough not to stall the
                    # PE/ACT phase-in with its PSUM-read WAR hazard
V_MID = (8, 13)     # direct DVE reduces slotted into tree gaps
TREE_BATCHES = [[0, 1, 2], [4, 5, 6], [7, 9, 10], [11, 12, 14], [15]]
A_UNITS = [m for b in TREE_BATCHES for m in b]
K_WARM = 5          # warm matmuls (1 low + rest mid ~ covers DMA phase)


def _install_ntff_shim():
    """Best-effort: register the axon NTFF profile hook so BASS_TRACE=1
    produces hardware profiles.  Safe no-op when unavailable."""
    try:
        import types

        import antenv

        if "antenv.axon_hooks" in sys.modules:
            return
        import trn_agent_boot.trn_boot as tb

        mod = types.ModuleType("antenv.axon_hooks")
        _hook = [None]
        mod.set_axon_ntff_profile_hook = lambda h: _hook.__setitem__(0, h)
        mod.get_axon_ntff_profile_hook = lambda: _hook[0]
        sys.modules["antenv.axon_hooks"] = mod
        antenv.axon_hooks = mod
        mod.set_axon_ntff_profile_hook(
            tb._ntff_profile_via_ctypes("/opt/axon/libaxon_pjrt.so")
        )
    except Exception:
        pass


def _build():
    nc = bacc.Bacc("TRN2", target_bir_lowering=False, debug=False, num_devices=NC)
    qT = nc.dram_tensor("qT", [D, BN], MMDT, kind="ExternalInput")
    dT = nc.dram_tensor("dT", [D, DCOLS], MMDT, kind="ExternalInput")
    nT = nc.dram_tensor("nT", [D, DCOLS], MMDT, kind="ExternalInput")
    qp = nc.dram_tensor("qp", [D, CL * N], MMDT, kind="ExternalInput")
    out_d = nc.dram_tensor("out", [128, 132], F16, kind="ExternalOutput")

    X = mybir.AxisListType.X

    with TileContext(nc) as tc:
        with (
            tc.tile_pool(name="sb", bufs=1) as sb,
            tc.tile_pool(name="ar", bufs=3) as arp,
            tc.tile_pool(name="tr", bufs=2) as trp,
            tc.tile_pool(name="ps", bufs=2, space="PSUM") as ps,
        ):
            qs = sb.tile([D, BN], MMDT, tag="qs")
            ds = sb.tile([D, DCOLS], MMDT, tag="ds")
            ns = sb.tile([D, DCOLS], MMDT, tag="ns")
            qps = sb.tile([D, CL * N], MMDT, tag="qps")
            maxall = sb.tile([128, 132], F16, tag="maxall")

            # Warm-up chain: memset-backed matmuls, no DMA dependency, keep
            # the PE continuously busy so the p-state ramp finishes before
            # the first real matmul.
            wa = sb.tile([D, 128], F16, tag="wa")
            wb = sb.tile([D, 512], F16, tag="wb")
            nc.gpsimd.memset(wa[:, :], 0.0)
            nc.gpsimd.memset(wb[:, :], 0.0)
            wt = ps.tile([128, 2048], F32, tag="chunk", name="warm")
            for w in range(K_WARM):
                nc.tensor.matmul(
                    wt[:, 512 * (w % 4) : 512 * (w % 4 + 1)],
                    wa[:, :],
                    wb[:, :],
                    start=True,
                    stop=True,
                )

            # Dummy 1-elem scalar copy: forces the implicit ACT_TABLE_LOAD
            # to be inserted here (runs during the DMA phase), so the first
            # real PSUM copy isn't delayed by the 1.28us table load.
            nc.scalar.copy(wb[0:1, 0:1], wa[0:1, 0:1])

            # Input DMAs, priority order.  The HWDGE queues (sync/SP and
            # scalar/ACT) are much faster to issue than gpsimd's SWDGE, so
            # the critical tensors (first query chunks + dT) go there; only
            # the tiny qp rides SWDGE.
            nc.sync.dma_start(out=qs[:, 0:512], in_=qT[:, 0:512])
            nc.scalar.dma_start(out=ds[:, 0:512], in_=dT[:, 0:512])
            nc.sync.dma_start(out=ds[:, 512:1024], in_=dT[:, 512:1024])
            nc.scalar.dma_start(out=ds[:, 1024:2048], in_=dT[:, 1024:2048])
            nc.sync.dma_start(out=qs[:, 512:1024], in_=qT[:, 512:1024])
            nc.sync.dma_start(out=ns[:, :], in_=nT[:, :])
            nc.gpsimd.dma_start(out=qps[:, :], in_=qp[:, :])
            nc.sync.dma_start(out=qs[:, 1024:2048], in_=qT[:, 1024:2048])

            arenas = {}
            bat_of = {}
            for bi, bb in enumerate(TREE_BATCHES):
                for sl, mm in enumerate(bb):
                    bat_of[mm] = (bi, sl)

            def get_arena(bi):
                if bi not in arenas:
                    arenas[bi] = arp.tile(
                        [128, 8192], F16, tag="arena", name=f"a{bi}"
                    )
                return arenas[bi]

            def emit_mms(m, t):
                bi, slot = bat_of.get(m, (None, None))
                for u in range(4):
                    nc.tensor.matmul(
                        t[:, 512 * u : 512 * (u + 1)],
                        qs[:, 128 * m : 128 * (m + 1)],
                        ds[:, 512 * u : 512 * (u + 1)],
                        start=True,
                        stop=True,
                    )
                    if m == V_HEAD and u == 1:
                        nc.vector.reduce_max(
                            maxall[:, 8 * m : 8 * m + 4],
                            t[:, 0:1024].rearrange("p (g s) -> p g s", s=S),
                            axis=X,
                        )
                    if m == A_SPLIT and u == 1:
                        nc.scalar.copy(
                            get_arena(bi)[:, 2048 * slot : 2048 * slot + 1024],
                            t[:, 0:1024],
                        )

            def emit_unit(m):
                t = ps.tile([128, 2048], F32, tag="chunk", name=f"u{m}")
                emit_mms(m, t)
                if m == V_HEAD:
                    nc.vector.reduce_max(
                        maxall[:, 8 * m + 4 : 8 * m + 8],
                        t[:, 1024:2048].rearrange("p (g s) -> p g s", s=S),
                        axis=X,
                    )
                elif m in V_MID:
                    nc.vector.reduce_max(
                        maxall[:, 8 * m : 8 * m + 8],
                        t[:, :].rearrange("p (g s) -> p g s", s=S),
                        axis=X,
                    )
                elif m == A_SPLIT:
                    bi, slot = bat_of[m]
                    nc.scalar.copy(
                        get_arena(bi)[:, 2048 * slot + 1024 : 2048 * (slot + 1)],
                        t[:, 1024:2048],
                    )
                else:
                    bi, slot = bat_of[m]
                    nc.scalar.copy(
                        get_arena(bi)[:, 2048 * slot : 2048 * (slot + 1)], t[:, :]
                    )

            s1outs = {}

            def emit_tree_s1(bi, sl):
                # stage 1 for one unit of batch bi (per-unit so the last
                # batch can start as each copy lands)
                arena = arenas[bi]
                if bi not in s1outs:
                    L = len(TREE_BATCHES[bi])
                    s1outs[bi] = trp.tile([128, 1024 * L], F16, tag="t1",
                                          name=f"t1_{bi}")
                gv = arena[:, 2048 * sl : 2048 * (sl + 1)].rearrange(
                    "p (g s) -> p g s", s=256
                )
                ov = s1outs[bi][:, 1024 * sl : 1024 * (sl + 1)].rearrange(
                    "p (g s) -> p g s", s=128
                )
                nc.vector.tensor_max(ov, gv[:, :, 0:128], gv[:, :, 128:256])

            def emit_tree_rest(bi):
                L = len(TREE_BATCHES[bi])
                t1v = s1outs[bi][:, 0 : 1024 * L].rearrange(
                    "p (g s) -> p g s", s=128
                )
                t2 = trp.tile([128, 2048], F16, tag="t2")
                t2v = t2[:, 0 : 512 * L].rearrange("p (g s) -> p g s", s=64)
                nc.vector.tensor_max(t2v, t1v[:, :, 0:64], t1v[:, :, 64:128])
                t3 = trp.tile([128, 1024], F16, tag="t3")
                t3v = t3[:, 0 : 256 * L].rearrange("p (g s) -> p g s", s=32)
                nc.vector.tensor_max(t3v, t2v[:, :, 0:32], t2v[:, :, 32:64])
                t4 = trp.tile([128, 512], F16, tag="t4")
                t4v = t4[:, 0 : 128 * L].rearrange("p (g s) -> p g s", s=16)
                nc.vector.tensor_max(t4v, t3v[:, :, 0:16], t3v[:, :, 16:32])
                for k, mm in enumerate(TREE_BATCHES[bi]):
                    nc.vector.reduce_max(
                        maxall[:, 8 * mm : 8 * mm + 8],
                        t4v[:, 8 * k : 8 * (k + 1), :],
                        axis=X,
                    )

            def emit_tree(bi):
                for sl in range(len(TREE_BATCHES[bi])):
                    emit_tree_s1(bi, sl)
                emit_tree_rest(bi)

            def emit_pairwise():
                pt = ps.tile([128, 2048], F32, tag="chunk", name="ptw")
                for b in range(CL):
                    g, j = divmod(b, 4)
                    nc.tensor.matmul(
                        pt[32 * j : 32 * (j + 1), 256 * g : 256 * (g + 1)],
                        qps[:, 32 * b : 32 * (b + 1)],
                        ns[:, 256 * b : 256 * (b + 1)],
                        start=True,
                        stop=True,
                        tile_position=(0, 32 * j),
                    )
                nc.vector.reduce_max(
                    maxall[:, 128:130],
                    pt[:, 0:512].rearrange("p (g s) -> p g s", s=S),
                    axis=X,
                )

            # ---- emission order (PE + engine queues are FIFO) ----------
            emit_unit(0)          # A-split: ACT starts after 2 of 4 MMs
            emit_unit(1)          # A
            emit_unit(2)          # A
            emit_pairwise()       # early DVE filler (needs ns+qps)
            emit_unit(3)          # Vh: halved DVE reduce
            emit_tree(0)          # B0 {0,1,2}
            for m in (4, 5, 6):
                emit_unit(m)
            emit_tree(1)          # B1 {4,5,6}
            for m in (7, 8):      # A, V-mid
                emit_unit(m)
            for m in (9, 10):
                emit_unit(m)
            emit_tree(2)          # B2 {7,9,10}
            nc.sync.dma_start(out=out_d[:, 0:64], in_=maxall[:, 0:64])
            emit_unit(11)
            emit_tree_s1(3, 0)
            emit_unit(12)
            emit_tree_s1(3, 1)
            emit_unit(13)         # V-mid fills DVE while c14 runs
            emit_unit(14)
            emit_tree_s1(3, 2)
            emit_tree_rest(3)     # B3 {11,12,14} closes before last copy
            emit_unit(15)
            emit_tree(4)          # solo {15}: ~1.5us tail after last copy
            nc.sync.dma_start(out=out_d[:, 64:132], in_=maxall[:, 64:132])

    nc.finalize()
    return nc


LAST_RESULT = None


def kernel(query_embeddings, doc_embeddings, neg_doc_embeddings):
    global LAST_RESULT
    _install_ntff_shim()

    q = np.asarray(query_embeddings, dtype=np.float32)
    d = np.asarray(doc_embeddings, dtype=np.float32)
    g = np.asarray(neg_doc_embeddings, dtype=np.float32)
    assert q.shape == (B, N, D) and d.shape == (B, S, D) and g.shape == (B, S, D)

    qT_all = np.ascontiguousarray(
        q.transpose(2, 0, 1).reshape(D, BN).astype(np.float16)
    )

    in_maps = []
    for k in range(NC):
        dT_k = np.ascontiguousarray(
            d[CL * k : CL * (k + 1)].transpose(2, 0, 1).reshape(D, DCOLS)
            .astype(np.float16)
        )
        nT_k = np.ascontiguousarray(
            g[CL * k : CL * (k + 1)].transpose(2, 0, 1).reshape(D, DCOLS)
            .astype(np.float16)
        )
        qp_k = np.ascontiguousarray(qT_all[:, CL * N * k : CL * N * (k + 1)])
        in_maps.append({"qT": qT_all, "dT": dT_k, "nT": nT_k, "qp": qp_k})

    if "nc" not in _CACHE:
        _CACHE["nc"] = _build()
    res = run_bass_kernel_spmd(_CACHE["nc"], in_maps, core_ids=list(range(NC)))
    LAST_RESULT = res

    # Host epilogue: n-sum, assembly, softplus means.
    scores = np.empty((B, B), dtype=np.float64)
    negpair = np.empty((B,), dtype=np.float64)
    for k in range(NC):
        o = res.results[k]["out"].astype(np.float64)  # (128, 132)
        for m in range(16):
            # rows 32j+n, block col 8m+c  ->  scores[4m+j, CL*k+c]
            blk = o[:, 8 * m : 8 * m + 8].reshape(4, N, CL).sum(axis=1)
            scores[4 * m : 4 * m + 4, CL * k : CL * (k + 1)] = blk
        pw = o[:, 128:130].reshape(4, N, 2).sum(axis=1)  # rows j, col g
        for gcol in range(2):
            for j in range(4):
                negpair[CL * k + 4 * gcol + j] = pw[j, gcol]

    pos = np.diagonal(scores)
    l1 = np.logaddexp(0.0, negpair - pos).mean()
    neg_ib = (scores - np.eye(B, dtype=np.float64) * NEG_INF_DIAG).max(axis=1)
    l2 = np.logaddexp(0.0, neg_ib - pos).mean()
    return np.asarray((l1 + l2) / 2.0, dtype=np.float32)
